# revision 31
# baseline (speedup 1.0000x reference)
"""Trainium2 Bass kernel for a GAT block.

Math (after algebraic simplification of the reference):
  h[b,f,n,k] = x[b,:,f,n] @ W[:,k] + bW[k]
  s2[b,f,n]  = h[b,f,n,:] @ a2 = v.x + c0   (s1/ab cancel inside softmax)
  d[b,f,n]   = softmax_n(s2)[n] * mask[n,n]
  out[b,k,f,n] = d[b,f,n] * h[b,f,n,k] = sum_c W[c,k] (x*d)[c,f,n] + bW[k] d[f,n]

Sharding: data-parallel over batch, 4 batches per core on 8 cores.

Device pipeline per (batch, 512-frame q-unit), shapes are [partitions, free]:
  1. xs [128, 3, 100]: partition = 4-frame quad (all 128 lanes busy)
  2. softmax on DVE/ACT -> dd128 [128, 100]; flatten DMA -> dd [32, 400]
  3. psum_dd [128, 400] = rep4.T @ dd   (PE replicates dd into 4 blocks)
  4. x4 [128, 400]: rows 32c+fsub = x[c], rows 96:128 = 1.0 (memset);
     x4s = x4 * psum_dd  (one DVE op: x*d rows 0:96, d rows 96:128)
  5. 16 matmuls into 2-bank psum tiles [128, 1024] (cols 0:400 and
     512:912): psum = wsel[tp].T @ x4s; wsel[tp] [128,128] selects fsubs
     {tp, 16+tp} and applies [W; bW] -> final out for 32 frames,
     rows = (2k + jj), cols (f', n)
  6. evict 2 tiles per op (DVE/ACT alternating) -> osb [128, 16, 400];
     2 half stores per q-unit ([128, 3200], 12.8KB descriptors)
"""

import sys

if "/opt/trn_rl_repo" not in sys.path:
    sys.path.insert(0, "/opt/trn_rl_repo")

import numpy as np

B, C, F, N, H = 32, 3, 2048, 25, 64
NCORES = 8
BPC = B // NCORES   # batches per core
QF = 512            # frames per q-unit
NQ = F // QF        # q-units per batch
FSUB = 16           # frames per fsub row
NS = QF // FSUB     # 32 fsub rows per q-unit
FN = F * N
TW = FSUB * N       # 400, columns per tile
NT = NS // 2        # 16 tiles (of 32 frames) per q-unit
QW = 4 * N          # 100, columns per frame-quad row

# matmul operand dtype: "f32" (exact) or "f32r" (~2e-4 rel err, 4x faster PE)
MM_DTYPE = "f32"

_NC_CACHE = {}


def _build_nc():
    import concourse.bass as bass
    import concourse.bacc as bacc
    import concourse.tile as tile
    from concourse import mybir

    f32 = mybir.dt.float32
    mmdt = f32 if MM_DTYPE == "f32" else mybir.dt.float32r
    MULT = mybir.AluOpType.mult
    ADD = mybir.AluOpType.add
    AX = mybir.AxisListType.X
    EXP = mybir.ActivationFunctionType.Exp

    nc = bacc.Bacc()
    x_d = nc.declare_dram_parameter("x", [BPC, C, F, N], f32, isOutput=False)
    wsel_d = nc.declare_dram_parameter("wsel", [128, NT, 128], mmdt, isOutput=False)
    rep4_d = nc.declare_dram_parameter("rep4", [NS, 128], f32, isOutput=False)
    v_d = nc.declare_dram_parameter("v_pp", [128, C], f32, isOutput=False)
    c0_d = nc.declare_dram_parameter("c0_pp", [128, 1], f32, isOutput=False)
    md_d = nc.declare_dram_parameter("mdq", [128, QW], f32, isOutput=False)
    out_d = nc.declare_dram_parameter("out", [BPC, H, F, N], f32, isOutput=True)

    with tile.TileContext(nc) as tc:
        with (
            tc.tile_pool(name="singles", bufs=1) as singles,
            tc.tile_pool(name="xs", bufs=3) as xs_pool,
            tc.tile_pool(name="sm", bufs=3) as sm_pool,
            tc.tile_pool(name="x4", bufs=3) as x4_pool,
            tc.tile_pool(name="osb", bufs=3) as osb_pool,
            tc.tile_pool(name="ps", bufs=7, space="PSUM") as ps_pool,
            tc.tile_pool(name="psd", bufs=1, space="PSUM") as psd_pool,
        ):
            wsel_sb = singles.tile([128, NT, 128], mmdt)
            nc.sync.dma_start(out=wsel_sb[:], in_=wsel_d[:, :, :])
            rep4_sb = singles.tile([NS, 128], f32)
            nc.sync.dma_start(out=rep4_sb[:], in_=rep4_d[:, :])
            v_sb = singles.tile([128, C], f32)
            nc.sync.dma_start(out=v_sb[:], in_=v_d[:, :])
            c0_sb = singles.tile([128, 1], f32)
            nc.sync.dma_start(out=c0_sb[:], in_=c0_d[:, :])
            md_sb = singles.tile([128, QW], f32)
            nc.sync.dma_start(out=md_sb[:], in_=md_d[:, :])

            units = [(b, q) for b in range(BPC) for q in range(NQ)]

            def emit_loads(u):
                """Emit the two input DMAs for unit u; return (xs, x4)."""
                b, q = u
                f0 = q * QF
                base = x_d[b, :, f0 : f0 + 1, :]  # for offset only
                xs = xs_pool.tile([128, C, QW], f32)
                src = bass.AP(
                    tensor=base.tensor,
                    offset=base.offset,
                    ap=[[QW, 128], [FN, C], [1, QW]],
                )
                nc.scalar.dma_start(out=xs[:], in_=src)
                x4 = x4_pool.tile([128, TW], f32, tag="x4")
                nc.vector.memset(x4[96:128, :], 1.0)
                src4 = bass.AP(
                    tensor=base.tensor,
                    offset=base.offset,
                    ap=[[FN, C], [TW, NS], [1, TW]],
                )
                nc.sync.dma_start(out=x4[0:96, :], in_=src4)
                return xs, x4

            pending = emit_loads(units[0])
            for ui, u in enumerate(units):
                b, q = u
                f0 = q * QF
                xs, x4 = pending
                if ui + 1 < len(units):
                    pending = emit_loads(units[ui + 1])
                # ---- 2. softmax in frame-quad layout -> dd128 [128, 100]
                t = sm_pool.tile([128, QW], f32, tag="t")
                nc.vector.tensor_scalar(
                    out=t[:],
                    in0=xs[:, 2, :],
                    scalar1=v_sb[:, 2:3],
                    scalar2=c0_sb[:, :],
                    op0=MULT,
                    op1=ADD,
                )
                for c in (1, 0):
                    nc.vector.scalar_tensor_tensor(
                        out=t[:],
                        in0=xs[:, c, :],
                        scalar=v_sb[:, c : c + 1],
                        in1=t[:],
                        op0=MULT,
                        op1=ADD,
                    )
                e = sm_pool.tile([128, QW], f32, tag="e")
                nc.scalar.activation(out=e[:], in_=t[:], func=EXP)
                ev = e[:].rearrange("p (a b) -> p a b", b=N)
                z = sm_pool.tile([128, 4], f32, tag="z")
                nc.vector.reduce_sum(out=z[:], in_=ev, axis=AX)
                r = sm_pool.tile([128, 4], f32, tag="r")
                nc.vector.reciprocal(out=r[:], in_=z[:])
                em = sm_pool.tile([128, QW], f32, tag="em")
                nc.vector.tensor_tensor(out=em[:], in0=e[:], in1=md_sb[:], op=MULT)
                dd128 = sm_pool.tile([128, QW], f32, tag="dd128")
                rr = r[:, :]
                r_bc = bass.AP(
                    tensor=rr.tensor,
                    offset=rr.offset,
                    ap=[rr.ap[0], [1, 4], [0, N]],
                )
                nc.vector.tensor_tensor(out=dd128[:], in0=em[:], in1=r_bc, op=MULT)
                # flatten [128, 100] -> [32, 400]
                dd = sm_pool.tile([NS, TW], f32, tag="dd")
                ddv = dd[:, :]
                dst = bass.AP(
                    tensor=ddv.tensor,
                    offset=ddv.offset,
                    ap=[ddv.ap[0], [QW, 4], [1, QW]],
                )
                nc.scalar.dma_start(out=dst, in_=dd128[:])
                # ---- 3. psum_dd [128, 400] = rep4.T @ dd
                pdd = psd_pool.tile([128, TW], f32, tag="pdd")
                nc.tensor.matmul(
                    pdd[:, :], rep4_sb[:], dd[:], start=True, stop=True
                )
                # ---- 4. x4s = x4 * psum_dd
                x4s = x4_pool.tile([128, TW], mmdt, tag="x4s")
                nc.vector.tensor_tensor(
                    out=x4s[:], in0=x4[:], in1=pdd[:], op=MULT
                )
                # ---- 5./6. 16 matmuls + evictions + stores
                osb = osb_pool.tile([128, NT, TW], f32)
                for tp in range(NT):
                    ph = ps_pool.tile([128, TW], f32, tag="ph")
                    nc.tensor.matmul(
                        ph[:, :],
                        wsel_sb[:, tp, :],
                        x4s[:, :],
                        start=True,
                        stop=True,
                    )
                    if tp % 3 == 0:
                        nc.vector.tensor_copy(osb[:, tp, :], ph[:, :])
                    else:
                        nc.scalar.copy(osb[:, tp, :], ph[:, :])
                    if tp % 8 == 7:
                        hh = tp // 8
                        osl = out_d[b, :, f0 : f0 + 1, :]
                        dst = bass.AP(
                            tensor=osl.tensor,
                            offset=osl.offset + hh * 8 * TW,
                            ap=[[FN, H], [16 * TW, 2], [1, 8 * TW]],
                        )
                        eng = nc.sync if hh == 0 else nc.scalar
                        eng.dma_start(
                            out=dst,
                            in_=osb[:, 8 * hh : 8 * (hh + 1), :],
                        )
    nc.compile()
    return nc


def _get_nc():
    if "nc" not in _NC_CACHE:
        _NC_CACHE["nc"] = _build_nc()
    return _NC_CACHE["nc"]


def _make_in_maps(x, mask, W, bW, a1, a2, ab):
    x = np.ascontiguousarray(np.asarray(x, np.float32))
    mask = np.asarray(mask, np.float32)
    W = np.asarray(W, np.float32)
    bW = np.asarray(bW, np.float32)
    a2 = np.asarray(a2, np.float32)

    v = (W @ a2).astype(np.float32)                    # [C]
    c0 = np.float32(bW @ a2)
    md = np.diag(mask).astype(np.float32)              # [N]

    # wsel[row = 32 c + fsub, tp, col = 2 k + jj]:
    #   delta[fsub == tp + 16 jj] * (W[c, k] if c < 3 else bW[k])
    # (column order (k, jj)-interleaved so the store DMA is affine)
    wsel = np.zeros((128, NT, 128), np.float32)
    cols = np.arange(H)
    for tp in range(NT):
        for jj in range(2):
            fsub = tp + 16 * jj
            for c in range(3):
                wsel[32 * c + fsub, tp, 2 * cols + jj] = W[c]
            wsel[96 + fsub, tp, 2 * cols + jj] = bW
    rep4 = np.zeros((NS, 128), np.float32)
    for blk in range(4):
        rep4[:, 32 * blk : 32 * (blk + 1)] = np.eye(NS, dtype=np.float32)
    v_pp = np.tile(v[None, :], (128, 1)).astype(np.float32)
    c0_pp = np.full((128, 1), c0, np.float32)
    mdq = np.tile(md[None, :], (128, 4)).astype(np.float32)

    in_maps = []
    for cix in range(NCORES):
        in_maps.append(
            {
                "x": np.ascontiguousarray(x[cix * BPC : (cix + 1) * BPC]),
                "wsel": wsel,
                "rep4": rep4,
                "v_pp": v_pp,
                "c0_pp": c0_pp,
                "mdq": mdq,
            }
        )
    return in_maps


def run(x, mask, W, bW, a1, a2, ab, **run_kwargs):
    from concourse.bass_utils import run_bass_kernel_spmd

    nc = _get_nc()
    in_maps = _make_in_maps(x, mask, W, bW, a1, a2, ab)
    res = run_bass_kernel_spmd(nc, in_maps, core_ids=list(range(NCORES)), **run_kwargs)
    out = np.concatenate([res.results[i]["out"] for i in range(NCORES)], axis=0)
    return out, res


def kernel(x, mask, W, bW, a1, a2, ab):
    out, _ = run(x, mask, W, bW, a1, a2, ab)
    return out
